# revision 1
# baseline (speedup 1.0000x reference)
"""BandSplit (BSRNN-style) Trainium2 kernel.

Reference computation (per batch sample, per band of width w, ch = 4w):
  h   = moveaxis(x[:, :, s:e, :, :], -1, 1).reshape(B, ch, T)   # channels (r, c, f)
  hn  = (h - mu) * rsqrt(var + eps) * gamma + beta              # GroupNorm(1, ch) over (ch, T)
  y   = W_band @ hn + b_band                                    # [128, T]
  out = stack over bands -> [B, 128, 31, T]

Folded form used here (r_b = rsqrt(var+eps); mu, r_b are per band+sample):
  y = r_b * (Wg @ h) + (v + b_band - r_b*mu*u)
  Wg = W * gamma (per column), u = Wg @ 1, v = W @ beta
so the big matmul runs on RAW h (no normalization pass over the data) and the
normalization is applied as a per-band scalar scale + per-output-channel bias
to the matmul output. Wg/u/v are parameter-only values and are prepared on the
host together with the weight layout packing; everything that touches x (the
matmuls, the mean/variance statistics, normalization, bias) runs on device.

Sharding: data-parallel over batch B=8 across the 8 NeuronCores (sample b on
core b); parameters replicated. Inside a core, h is never materialized:
matmuls read the natively-laid-out staged x tiles (partitions = (c, f) rows,
free = interleaved (t, r)) with a stride-2 free-dim access pattern selecting
the real/imag plane. Per-band sums come from one-hot-stationary matmuls
(partition reduction on the PE); sums of squares from scalar-engine
Square+accumulate passes. x is staged in multi-band "super tiles" so each
DMA moves >= ~1 MB (wide hardware-queue fan-out).
"""

import numpy as np

import concourse.bass as bass
import concourse.tile as tile
from concourse import bacc, mybir

F32 = mybir.dt.float32
F32R = mybir.dt.float32r
AFT = mybir.ActivationFunctionType
ALU = mybir.AluOpType

# ---------------------------------------------------------------- problem dims
WIDTHS = [25] * 10 + [50] * 12 + [100] * 8 + [399]
NBANDS = len(WIDTHS)          # 31
C_IN = 2
T = 512
OUT_CH = 128
EPS = 1e-5
F_TOT = 2049
N_CORES = 8
N_WTP_PIECES = 4
OUT_GROUPS = [(0, 8), (8, 16), (16, 24), (24, 31)]

_STARTS = np.concatenate([[0], np.cumsum(WIDTHS)]).astype(int)
_CHOFF = np.concatenate([[0], np.cumsum([4 * w for w in WIDTHS])]).astype(int)


def _tables():
    """Super-tile staging plan + weight-chunk slots + S2 strip columns.

    Each super tile is one DMA; its free dim indexes "subs". A sub is one
    former staged tile: partitions = (c, f) rows of one band (c-stacked when
    2w <= 128), free row = interleaved (t, r) of 1024 floats. Each sub feeds
    two weight-chunk slots (one per r-plane), each a contiguous channel range
    (channel index within band = r*2w + c*w + f).
    """
    supers = []  # dict: kind, bands/c info for DMA AP, subs: [(band, K, [(a,e),(a,e)])]

    def band_sub(i, w, off):
        # c-stacked sub for a 2w<=128 band
        return (i, 2 * w, [(off, off + 2 * w), (off + 2 * w, off + 4 * w)])

    # class A: w=25 bands 0..9, pairs; band b's rows at partition offset 64
    # (spreads DMA traffic across more engines; the two bands' matmuls run in
    # different PE row-groups concurrently)
    for a in range(5):
        i0 = 2 * a
        supers.append(dict(
            kind="pair64", f0=int(_STARTS[i0]), w=25,
            subs=[band_sub(i0 + j, 25, int(_CHOFF[i0 + j])) for j in range(2)],
            p0=[0, 64],
        ))
    # class B: w=50 bands 10..21, pairs
    for a in range(6):
        i0 = 10 + 2 * a
        supers.append(dict(
            kind="pair", f0=int(_STARTS[i0]), w=50,
            subs=[band_sub(i0 + j, 50, int(_CHOFF[i0 + j])) for j in range(2)],
        ))
    # class C: w=100 bands 22..29, one super per band, subs = (c0, c1)
    for i in range(22, 30):
        off = int(_CHOFF[i])
        w = 100
        supers.append(dict(
            kind="cpair", f0=int(_STARTS[i]), w=w, nf=w,
            subs=[(i, w, [(off + r * 2 * w + c * w, off + r * 2 * w + (c + 1) * w)
                          for r in range(2)]) for c in range(2)],
        ))
    # class D: band 30 (w=399), supers per f-chunk, subs = (c0, c1)
    i = 30
    off = int(_CHOFF[i])
    w = 399
    for f0 in range(0, w, 128):
        f1 = min(f0 + 128, w)
        supers.append(dict(
            kind="cpair", f0=int(_STARTS[i]) + f0, w=w, nf=f1 - f0,
            subs=[(i, f1 - f0,
                   [(off + r * 2 * w + c * w + f0, off + r * 2 * w + c * w + f1)
                    for r in range(2)]) for c in range(2)],
        ))

    # weight slots: one per (super, sub, r), in traversal order
    slots = []
    for si, sup in enumerate(supers):
        p0s = sup.get("p0", [0] * len(sup["subs"]))
        span = max(p0s[j] + sup["subs"][j][1] for j in range(len(sup["subs"])))
        for j, (band, K, chs) in enumerate(sup["subs"]):
            for r in range(2):
                slots.append(dict(super=si, sub=j, r=r, ch=chs[r], p0=p0s[j],
                                  span=span))

    # S2 strip columns: one ACT op per (super, band-different subs) or per super
    # (when both subs are the same band). Per-band ranges padded to EVEN width.
    s2ops = []  # (super_idx, sub_list, band, col)
    band_ncols = [0] * NBANDS
    per_super_ops = []
    for si, sup in enumerate(supers):
        bands = {b for (b, _, _) in sup["subs"]}
        if len(bands) == 1:
            per_super_ops.append((si, list(range(len(sup["subs"]))), sup["subs"][0][0]))
        else:
            for j, (band, K, _) in enumerate(sup["subs"]):
                per_super_ops.append((si, [j], band))
    band_cols = [[] for _ in range(NBANDS)]
    col = 0
    for band in range(NBANDS):
        ops_b = [(si, js) for (si, js, b) in per_super_ops if b == band]
        for k, (si, js) in enumerate(ops_b):
            s2ops.append((si, js, band, col + k))
        width = len(ops_b) + (len(ops_b) % 2)
        band_cols[band] = list(range(col, col + width))
        col += width
    return supers, slots, s2ops, band_cols, int(col)


SUPERS, SLOTS, S2OPS, BAND_S2COLS, N_S2COLS = _tables()
N_SLOTS = len(SLOTS)  # 92


def _wtp_pieces():
    """Group weight slots into DMA pieces of equal partition span (p0 + K).
    Returns list of (span, [slot indices])."""
    pieces = []
    cur = []
    cur_k = None
    for j, sl in enumerate(SLOTS):
        K = sl["ch"][1] - sl["ch"][0]
        span = sl["span"]
        Kc = 128 if span > 100 else span
        if K == 15:
            Kc = 15
        if cur_k is None or Kc != cur_k or len(cur) >= 24:
            if cur:
                pieces.append((cur_k, cur))
            cur = []
            cur_k = Kc
        cur.append(j)
    if cur:
        pieces.append((cur_k, cur))
    return pieces


WTP_PIECES = _wtp_pieces()


def _pack_params(W, gamma, beta, bb):
    """Host-side preparation of the parameter-only tensors."""
    Wg = (W * gamma[None, :]).astype(np.float32)
    WgT = np.ascontiguousarray(Wg.T)
    # concatenated class-packed pieces: piece (Kp, js) occupies Kp * len(js) * 128
    total = sum(Kp * len(js) * 128 for (Kp, js) in WTP_PIECES)
    wtp = np.zeros((total,), np.float32)
    off = 0
    for (Kp, js) in WTP_PIECES:
        blk = np.zeros((Kp, len(js), 128), np.float32)
        for k, j in enumerate(js):
            a, e = SLOTS[j]["ch"]
            p0 = SLOTS[j]["p0"]
            blk[p0: p0 + e - a, k, :] = WgT[a:e, :]
        n = blk.size
        wtp[off:off + n] = blk.reshape(-1)
        off += n
    wtp = wtp.reshape(1, -1)
    uvb = np.zeros((128, 2, NBANDS), np.float32)
    for i in range(NBANDS):
        a, e = int(_CHOFF[i]), int(_CHOFF[i + 1])
        uvb[:, 0, i] = Wg[:, a:e].sum(axis=1)
        uvb[:, 1, i] = W[:, a:e] @ beta[a:e] + bb[i]
    return wtp, uvb


def _super_dmas(nc, x_d, sup, xt):
    """Issue the staging DMA(s) for one super tile.

    pair supers need one DMA per c (the (c, f) partition dim is not a single
    stride), cpair supers are a single 3D AP.
    """
    xr = x_d.bitcast(F32R)
    base = xr[0, 0, 0, 0]
    CS = F_TOT * T * 2          # c stride (elements)
    FS = T * 2                  # f stride
    off = sup["f0"] * FS
    if sup["kind"] == "pair64":
        w = sup["w"]
        for j in range(2):  # band half
            for c in range(2):
                p0 = sup["p0"][j] + c * w
                ap = bass.AP(
                    tensor=base.tensor,
                    offset=base.offset + off + j * w * FS + c * CS,
                    ap=[[FS, w], [1, 1024]])
                nc.sync.dma_start(out=xt[p0: p0 + w, j, :], in_=ap)
    elif sup["kind"] == "pair":
        w = sup["w"]
        for c in range(2):
            ap = bass.AP(tensor=base.tensor, offset=base.offset + off + c * CS,
                         ap=[[FS, w], [FS * w, 2], [1, 1024]])
            nc.sync.dma_start(out=xt[c * w: (c + 1) * w, :, :], in_=ap)
    else:
        nf = sup["nf"]
        ap = bass.AP(tensor=base.tensor, offset=base.offset + off,
                     ap=[[FS, nf], [CS, 2], [1, 1024]])
        nc.sync.dma_start(out=xt[0:nf, :, :], in_=ap)


GROUPS = [  # (super index range, band range)
    ((0, 5), (0, 10)),     # class A
    ((5, 11), (10, 22)),   # class B
    ((11, 19), (22, 30)),  # class C
    ((19, 23), (30, 31)),  # class D
]


def _build_nc():
    nc = bacc.Bacc("TRN2")

    x_d = nc.dram_tensor("xb", [C_IN, F_TOT, T, 2], F32, kind="ExternalInput")
    wtp_total = sum(Kp * len(js) * 128 for (Kp, js) in WTP_PIECES)
    wtp_d = nc.dram_tensor("wtp", [1, wtp_total], F32, kind="ExternalInput")
    uvb_d = nc.dram_tensor("uvb", [128, 2, NBANDS], F32, kind="ExternalInput")
    y_d = nc.dram_tensor("y", [OUT_CH, NBANDS, T], F32, kind="ExternalOutput")

    # DRAM scratch for cross-partition broadcasts (written then read back)
    cvec_d = nc.dram_tensor("cvec_scratch", [1, NBANDS], F32)
    rpack_d = nc.dram_tensor("rpack_scratch", [NBANDS, 2], F32)

    # map super index -> group index, and slot traversal order
    sup_group = {}
    for gi, ((s0, s1), _) in enumerate(GROUPS):
        for si in range(s0, s1):
            sup_group[si] = gi

    with tile.TileContext(nc) as tc:
        with tc.tile_pool(name="persist", bufs=1) as persist, \
             tc.tile_pool(name="stage", bufs=12) as stage, \
             tc.tile_pool(name="wtpp", bufs=2) as wtpp, \
             tc.tile_pool(name="osbp", bufs=2) as osbp, \
             tc.tile_pool(name="grp", bufs=2) as grp, \
             tc.tile_pool(name="scratch", bufs=1) as scratchp, \
             tc.tile_pool(name="small", bufs=1) as small, \
             tc.tile_pool(name="psmain", bufs=4, space="PSUM") as psmain, \
             tc.tile_pool(name="pss1", bufs=2, space="PSUM") as pss1, \
             tc.tile_pool(name="pss2", bufs=2, space="PSUM") as pss2:

            # ------------- constants ----------------------------------------
            ohm32 = small.tile([128, 63], F32)
            nc.vector.memset(ohm32, 0.0)
            nc.vector.memset(ohm32[:, 31:32], 1.0)
            ohm = small.tile([128, 63], F32R)
            nc.vector.tensor_copy(out=ohm, in_=ohm32)

            cvec = small.tile([1, NBANDS], F32)
            for i, w in enumerate(WIDTHS):
                nc.vector.memset(cvec[0:1, i:i + 1], 1.0 / (4 * w * T))
            nc.scalar.dma_start(out=cvec_d[:], in_=cvec)

            epst = small.tile([16, 1], F32)
            nc.vector.memset(epst, EPS)

            strip = small.tile([128, N_S2COLS], F32)
            nc.vector.memset(strip, 0.0)

            uvb = persist.tile([128, 2, NBANDS], F32)
            nc.scalar.dma_start(out=uvb, in_=uvb_d[:])

            # wtp piece 0 on the sync queue (gates the very first matmuls),
            # the rest on the act queue.
            wtps = []
            slot_tile = [None] * N_SLOTS
            off = 0
            for p, (Kp, js) in enumerate(WTP_PIECES):
                ns = len(js)
                wt = wtpp.tile([128, ns, 128], F32R, tag="wtpc", name=f"wtp{p}")
                src = wtp_d.bitcast(F32R)[0, off: off + Kp * ns * 128]
                ap = bass.AP(tensor=src.tensor, offset=src.offset,
                             ap=[[ns * 128, Kp], [128, ns], [1, 128]])
                eng = nc.sync if p == 0 else nc.scalar
                eng.dma_start(out=wt[0:Kp, :, :], in_=ap)
                off += Kp * ns * 128
                wtps.append(wt)
                for k, j in enumerate(js):
                    slot_tile[j] = (p, k)

            # ------------- streaming over groups ----------------------------
            band_nmm = {}
            for sl in SLOTS:
                b = SUPERS[sl["super"]]["subs"][sl["sub"]][0]
                band_nmm[b] = band_nmm.get(b, 0) + 1

            slot_iter = 0
            for gi, ((s0, s1), (b0, b1)) in enumerate(GROUPS):
                ng = b1 - b0
                osb = osbp.tile([128, 12, T], F32, tag="osb", name=f"osb{gi}")
                s1g = pss1.tile([32, T], F32, tag="s1g", name=f"s1g{gi}")
                s2g = pss2.tile([32, 24], F32, tag="s2g", name=f"s2g{gi}")
                band_done = {b_: 0 for b_ in range(b0, b1)}
                band_psum = {}
                n_s1 = sum(2 * len(SUPERS[si]["subs"]) for si in range(s0, s1))
                s1_idx = 0
                sup_s2cols = {}

                for si in range(s0, s1):
                    sup = SUPERS[si]
                    nsub = len(sup["subs"])
                    xt = stage.tile([128, nsub, 1024], F32R, tag="xt",
                                    name=f"xt{si}")
                    _super_dmas(nc, x_d, sup, xt)

                    for j, (band, K, chs) in enumerate(sup["subs"]):
                        brel = band - b0
                        p0 = sup.get("p0", [0] * nsub)[j]
                        xv = xt[:, j, :].rearrange("p (t r) -> p t r", r=2)
                        if band not in band_psum:
                            band_psum[band] = psmain.tile(
                                [128, T], F32, tag="acc", name=f"acc{band}")
                        for r in range(2):
                            pi, lj = slot_tile[slot_iter]
                            slot_iter += 1
                            band_done[band] += 1
                            nc.tensor.matmul(
                                band_psum[band][:],
                                wtps[pi][p0:p0 + K, lj, :],
                                xv[p0:p0 + K, :, r],
                                start=(band_done[band] == 1),
                                stop=(band_done[band] == band_nmm[band]),
                            )
                        for h in range(2):
                            s1_idx += 1
                            nc.tensor.matmul(
                                s1g[:],
                                ohm[p0:p0 + K, 31 - brel: 63 - brel],
                                xt[p0:p0 + K, j, h * T: (h + 1) * T],
                                start=(s1_idx == 1),
                                stop=(s1_idx == n_s1),
                            )
                        if band_done[band] == band_nmm[band]:
                            acc = band_psum.pop(band)
                            nc.vector.tensor_copy(out=osb[:, brel, :],
                                                  in_=acc[:])

                    # S2 square+accumulate for this super (scalar engine)
                    for (ssi, js, band, col) in S2OPS:
                        if ssi != si:
                            continue
                        K = sup["subs"][js[0]][1]
                        q0 = sup.get("p0", [0] * nsub)[js[0]]
                        sq = scratchp.tile([128, 2048], F32, tag="sq",
                                           name=f"sq{si}_{js[0]}")
                        if len(js) == 1:
                            in_ap = xt.bitcast(F32)[q0:q0 + K, js[0], :]
                            out_ap = sq[q0:q0 + K, 0:1024]
                        else:
                            in_ap = xt.bitcast(F32)[q0:q0 + K, :, :]
                            out_ap = sq[q0:q0 + K, 0: 1024 * len(js)]
                        nc.scalar.activation(
                            out=out_ap, in_=in_ap, func=AFT.Square,
                            accum_out=strip[q0:q0 + K, col: col + 1],
                        )

                # ---- group statistics ----
                strip_cols = [c for b_ in range(b0, b1) for c in BAND_S2COLS[b_]]
                gc0, gc1 = strip_cols[0], strip_cols[-1] + 1
                strip_r = grp.tile([128, 24], F32R, tag="stripr", name=f"str{gi}")
                nc.vector.tensor_copy(out=strip_r[:, 0: gc1 - gc0],
                                      in_=strip[:, gc0:gc1])
                for k, band in enumerate(range(b0, b1)):
                    cols = BAND_S2COLS[band]
                    c0, c1 = cols[0] - gc0, cols[-1] + 1 - gc0
                    brel = band - b0
                    nc.tensor.matmul(
                        s2g[:, c0:c1],
                        ohm[0:128, 31 - brel: 63 - brel],
                        strip_r[:, c0:c1],
                        start=(k == 0), stop=(k == ng - 1),
                    )

                s1red = grp.tile([16, 1], F32, tag="s1red", name=f"s1r{gi}")
                nc.vector.tensor_reduce(out=s1red[0:ng, :], in_=s1g[0:ng, :],
                                        axis=mybir.AxisListType.X, op=ALU.add)
                s2red = grp.tile([16, 1], F32, tag="s2red", name=f"s2r{gi}")
                nc.vector.tensor_reduce(out=s2red[0:ng, :],
                                        in_=s2g[0:ng, 0: gc1 - gc0],
                                        axis=mybir.AxisListType.X, op=ALU.add)

                invn = grp.tile([16, 1], F32, tag="invn", name=f"inv{gi}")
                src = cvec_d[0:1, b0:b1]
                nc.scalar.dma_start(
                    out=invn[0:ng, :],
                    in_=bass.AP(tensor=src.tensor, offset=src.offset,
                                ap=[[1, ng], [1, 1]]),
                )

                mu = grp.tile([16, 1], F32, tag="mu", name=f"mu{gi}")
                nc.vector.tensor_mul(out=mu[0:ng], in0=s1red[0:ng],
                                     in1=invn[0:ng])
                ex2 = grp.tile([16, 1], F32, tag="ex2", name=f"ex2{gi}")
                nc.vector.tensor_mul(out=ex2[0:ng], in0=s2red[0:ng],
                                     in1=invn[0:ng])
                musq = grp.tile([16, 1], F32, tag="musq", name=f"msq{gi}")
                nc.vector.tensor_mul(out=musq[0:ng], in0=mu[0:ng], in1=mu[0:ng])
                var = grp.tile([16, 1], F32, tag="var", name=f"var{gi}")
                nc.vector.tensor_tensor(out=var[0:ng], in0=ex2[0:ng],
                                        in1=musq[0:ng], op=ALU.subtract)
                std = grp.tile([16, 1], F32, tag="std", name=f"std{gi}")
                nc.scalar.activation(out=std[0:ng], in_=var[0:ng],
                                     func=AFT.Sqrt, bias=epst[0:ng, 0:1])
                rpack = grp.tile([16, 2], F32, tag="rpack", name=f"rp{gi}")
                nc.vector.reciprocal(out=rpack[0:ng, 0:1], in_=std[0:ng])
                nc.vector.tensor_mul(out=rpack[0:ng, 1:2], in0=rpack[0:ng, 0:1],
                                     in1=mu[0:ng])

                nc.scalar.dma_start(out=rpack_d[b0:b1, :], in_=rpack[0:ng, :])
                rbu = grp.tile([128, 12, 2], F32, tag="rbu", name=f"rbu{gi}")
                src_r = rpack_d[b0:b0 + 1, 0:1]
                nc.scalar.dma_start(
                    out=rbu[:, 0:ng, :],
                    in_=bass.AP(tensor=src_r.tensor, offset=src_r.offset,
                                ap=[[0, 128], [2, ng], [1, 2]]),
                )

                t_ru = grp.tile([128, 12], F32, tag="tru", name=f"tru{gi}")
                nc.vector.tensor_mul(out=t_ru[:, 0:ng], in0=rbu[:, 0:ng, 1],
                                     in1=uvb[:, 0, b0:b1])
                bbv = grp.tile([128, 12], F32, tag="bbv", name=f"bbv{gi}")
                nc.vector.tensor_tensor(out=bbv[:, 0:ng], in0=uvb[:, 1, b0:b1],
                                        in1=t_ru[:, 0:ng], op=ALU.subtract)

                # finalize in place (split DVE/ACT) + one grouped store
                for brel in range(ng):
                    if brel % 2 == 0:
                        nc.vector.tensor_scalar(
                            out=osb[:, brel, :], in0=osb[:, brel, :],
                            scalar1=rbu[:, brel, 0:1],
                            scalar2=bbv[:, brel: brel + 1],
                            op0=ALU.mult, op1=ALU.add,
                        )
                    else:
                        nc.scalar.activation(
                            out=osb[:, brel, :], in_=osb[:, brel, :],
                            func=AFT.Identity,
                            scale=rbu[:, brel, 0:1],
                            bias=bbv[:, brel: brel + 1],
                        )
                nc.scalar.dma_start(out=y_d[:, b0:b1, :], in_=osb[:, 0:ng, :])

    nc.finalize()
    return nc


_NC_CACHE = None


def _get_nc():
    global _NC_CACHE
    if _NC_CACHE is None:
        _NC_CACHE = _build_nc()
    return _NC_CACHE


def kernel(x, gamma, beta, W, b):
    from concourse.bass_utils import run_bass_kernel_spmd

    x = np.asarray(x, dtype=np.float32)
    gamma = np.asarray(gamma, dtype=np.float32)
    beta = np.asarray(beta, dtype=np.float32)
    W = np.asarray(W, dtype=np.float32)
    b = np.asarray(b, dtype=np.float32)

    wtp, uvb = _pack_params(W, gamma, beta, b)
    nc = _get_nc()
    in_maps = [
        {"xb": np.ascontiguousarray(x[i]), "wtp": wtp, "uvb": uvb}
        for i in range(N_CORES)
    ]
    res = run_bass_kernel_spmd(nc, in_maps, list(range(N_CORES)))
    return np.stack([res.results[i]["y"] for i in range(N_CORES)], axis=0)



# revision 24
# speedup vs baseline: 1.6463x; 1.6463x over previous
"""BandSplit (BSRNN-style) Trainium2 kernel — fp16 channel-major rewrite.

Reference computation (per batch sample, per band of width w, ch = 4w):
  h   = moveaxis(x[:, :, s:e, :, :], -1, 1).reshape(B, ch, T)   # channels (r, c, f)
  hn  = (h - mu) * rsqrt(var + eps) * gamma + beta              # GroupNorm(1, ch) over (ch, T)
  y   = W_band @ hn + b_band                                    # [128, T]
  out = stack over bands -> [B, 128, 31, T]

Folded form used here (r_b = rsqrt(var+eps); mu, r_b per band+sample):
  y = r_b * (Wg @ h) + (v + b_band - r_b*mu*u),  Wg = W*gamma, u = Wg@1, v = W@beta
so the big matmul runs on RAW h and normalization is a per-band scalar scale +
per-output-channel bias on the matmul output.

Layout: the host de-interleaves x into a dense channel-major fp16 tensor
xp[p, t, :] = h[128*t + p, :] (bands concatenated in descending-width order,
124 pad slots at the end).  One core per batch sample; per core:
  - main matmuls: one per (band x 128-column) segment, K<=128, fp16
  - stats: one DVE bn_stats per column -> per-(p,t) count/mean/M2; converted to
    per-(p,t) (sum, sumsq) proxies; band sums via tiny per-column selector
    matmuls on the PE (data as stationary [128,2], 0/1 selector as moving)
  - normalization: scale+bias folded, applied in-place on fp16 output tiles
  - everything fp16 over the wire (x, W, output), fp32 accumulation in PSUM
"""

import numpy as np

import concourse.bass as bass
import concourse.tile as tile
from concourse import bacc, mybir

F32 = mybir.dt.float32
F16 = mybir.dt.float16
AFT = mybir.ActivationFunctionType
ALU = mybir.AluOpType

# ---------------------------------------------------------------- problem dims
WIDTHS = [25] * 10 + [50] * 12 + [100] * 8 + [399]
NBANDS = len(WIDTHS)          # 31
C_IN = 2
T = 512
OUT_CH = 128
EPS = 1e-5
F_TOT = 2049
N_CORES = 8
N_CH = sum(4 * w for w in WIDTHS)       # 8196

_CHOFF_NAT = np.concatenate([[0], np.cumsum([4 * w for w in WIDTHS])]).astype(int)

# packed band order: descending width (big band first so its stats/finalize
# overlap with later compute; smallest bands finish last with minimal tail)
# Band starts are 32-aligned so every matmul segment lands on a legal PE tile
# position (base 0/32/64/96 with the per-base K limits).
PACKED_BANDS = [30] + list(range(22, 30)) + list(range(10, 22)) + list(range(10))
_PS = []  # (start, end) per packed band, 32-aligned starts (never 96 mod 128)
_s = 0
for _b in PACKED_BANDS:
    _PS.append((_s, _s + 4 * WIDTHS[_b]))
    _s = -(-(_s + 4 * WIDTHS[_b]) // 32) * 32
    if _s % 128 == 96:
        _s += 32
N_COLS = (_PS[-1][1] + 127) // 128      # 70
N_SLOT = N_COLS * 128                   # 8960

# groups (ranges of packed band indices) for pipelined stats/finalize
GROUPS = [(0, 1), (1, 9), (9, 21), (21, 31)]


def _tables():
    # matmul segments: (t, p0, p1, packed_band), column-major order.
    # Split base-32 segments with K > 32 (illegal PE tile position).
    segs = []
    for pb in range(NBANDS):
        s, e = _PS[pb]
        for t in range(s // 128, (e - 1) // 128 + 1):
            p0 = max(s - 128 * t, 0)
            p1 = min(e - 128 * t, 128)
            if p0 == 32 and p1 - p0 > 32:
                segs.append((t, 32, 64, pb))
                segs.append((t, 64, p1, pb))
            else:
                segs.append((t, p0, p1, pb))
    segs.sort(key=lambda q: (q[0], q[1]))

    # per-group column ranges and selector columns (boundary cols duplicated)
    ginfo = []
    selcols = []  # (group, t) in emission order
    for gi, (b0, b1) in enumerate(GROUPS):
        ch0, ch1 = _PS[b0][0], _PS[b1 - 1][1]
        t0, t1 = ch0 // 128, (ch1 - 1) // 128 + 1
        k0 = len(selcols)
        for t in range(t0, t1):
            selcols.append((gi, t))
        ginfo.append(dict(b0=b0, b1=b1, t0=t0, t1=t1, k0=k0, k1=len(selcols)))
    return segs, ginfo, selcols


SEGS, GINFO, SELCOLS = _tables()
N_SEL = len(SELCOLS)


def _pack_params(W, gamma, beta, bb):
    """Host-side parameter packing (parameter-only; no x data touched)."""
    Wg = (W * gamma[None, :]).astype(np.float32)
    wt = np.zeros((N_SLOT, OUT_CH), np.float32)
    for pb, b in enumerate(PACKED_BANDS):
        s, e = _PS[pb]
        wt[s:e] = Wg.T[_CHOFF_NAT[b]:_CHOFF_NAT[b + 1]]
    wt = np.ascontiguousarray(
        wt.reshape(N_COLS, 128, OUT_CH).transpose(1, 0, 2)
    ).astype(np.float16)                             # [128, N_COLS, 128]

    uvb = np.zeros((OUT_CH, 2, NBANDS), np.float32)
    cc = np.zeros((32, 2 * len(GROUPS)), np.float32)
    for pb, b in enumerate(PACKED_BANDS):
        a, e = int(_CHOFF_NAT[b]), int(_CHOFF_NAT[b + 1])
        uvb[:, 0, pb] = Wg[:, a:e].sum(axis=1)
        uvb[:, 1, pb] = W[:, a:e] @ beta[a:e] + bb[b]
        n = (e - a) * T
        gi = next(i for i, (g0, g1) in enumerate(GROUPS) if g0 <= pb < g1)
        cc[pb - GROUPS[gi][0], 2 * gi] = 256.0 / n
        cc[pb - GROUPS[gi][0], 2 * gi + 1] = 1.0 / n

    # selector one-hots map channel (p, t) -> group-RELATIVE band row
    sel = np.zeros((128, N_SEL, 32), np.float16)
    band_of = np.full(N_SLOT, -1, np.int32)
    for pb in range(NBANDS):
        band_of[_PS[pb][0]:_PS[pb][1]] = pb
    for k, (gi, t) in enumerate(SELCOLS):
        b0, b1 = GROUPS[gi]
        ch = 128 * t + np.arange(128)
        j = band_of[ch]
        m = (j >= b0) & (j < b1)
        sel[np.arange(128)[m], k, j[m] - b0] = 1.0
    return wt, uvb, cc, sel


def _pack_x(x):
    """x [8, 2, 2049, 512, 2] fp32 -> [8, 128, N_COLS, 512] fp16 channel-major."""
    fstarts = np.concatenate([[0], np.cumsum(WIDTHS)]).astype(int)
    xr = x.transpose(0, 4, 1, 2, 3)                  # [B, r, c, F, T]
    xp = np.zeros((x.shape[0], N_SLOT, T), np.float16)
    for pb, b in enumerate(PACKED_BANDS):
        s, w = int(fstarts[b]), WIDTHS[b]
        xp[:, _PS[pb][0]:_PS[pb][1]] = \
            xr[:, :, :, s:s + w, :].reshape(x.shape[0], 4 * w, T)
    return np.ascontiguousarray(
        xp.reshape(x.shape[0], N_COLS, 128, T).transpose(0, 2, 1, 3)
    )                                                # [B, 128, N_COLS, T]


def _build_nc():
    nc = bacc.Bacc("TRN2")

    x_d = nc.dram_tensor("xp", [128, N_COLS, T], F16, kind="ExternalInput")
    wt_d = nc.dram_tensor("wt", [128, N_COLS, OUT_CH], F16, kind="ExternalInput")
    sel_d = nc.dram_tensor("sel", [128, N_SEL, 32], F16, kind="ExternalInput")
    uvb_d = nc.dram_tensor("uvb", [OUT_CH, 2, NBANDS], F32, kind="ExternalInput")
    cc_d = nc.dram_tensor("cc", [32, 2 * len(GROUPS)], F32, kind="ExternalInput")
    y_d = nc.dram_tensor("y", [OUT_CH, NBANDS, T], F16, kind="ExternalOutput")

    # map: for each column, the segments in it; band seg counts for start/stop
    col_segs = {}
    for (t, p0, p1, pb) in SEGS:
        col_segs.setdefault(t, []).append((p0, p1, pb))
    band_nseg = {}
    for (_, _, _, pb) in SEGS:
        band_nseg[pb] = band_nseg.get(pb, 0) + 1
    # group block is emitted after its last column's work
    gend = {g["t1"] - 1: gi for gi, g in enumerate(GINFO)}
    x_chunks = [(c, min(c + 8, N_COLS)) for c in range(0, N_COLS, 8)]
    chunk_start = {c0: (c0, c1) for (c0, c1) in x_chunks}

    with tile.TileContext(nc) as tc:
        with tc.tile_pool(name="pers", bufs=1) as pers, \
             tc.tile_pool(name="grp", bufs=2) as grp, \
             tc.tile_pool(name="psacc", bufs=5, space="PSUM") as psacc, \
             tc.tile_pool(name="pssel", bufs=2, space="PSUM") as pssel:

            xt = pers.tile([128, N_COLS, T], F16)
            wt = pers.tile([128, N_COLS, OUT_CH], F16)
            selp = pers.tile([128, N_SEL, 32], F16)
            uvb = pers.tile([OUT_CH, 2, NBANDS], F32)
            cc = pers.tile([32, 2 * len(GROUPS)], F32)
            osb = pers.tile([128, NBANDS, T], F16)
            s6 = pers.tile([128, N_COLS, 6], F16)
            s12m = pers.tile([128, N_COLS, 2], F16)
            epst = pers.tile([32, 1], F32)

            # parameter DMAs (scalar queue) + constants
            nc.scalar.dma_start(out=wt, in_=wt_d[:])
            nc.scalar.dma_start(out=selp, in_=sel_d[:])
            nc.scalar.dma_start(out=uvb, in_=uvb_d[:])
            nc.scalar.dma_start(out=cc, in_=cc_d[:])
            nc.vector.memset(epst, EPS)

            band_psum = {}
            band_done = {}
            copy_alt = [0]

            def do_col(t):
                # bn_stats for this column (DVE)
                nc.vector.bn_stats(out=s6[:, t, :], in_=xt[:, t, :])
                # main matmul segments
                for (p0, p1, pb) in col_segs.get(t, []):
                    if pb not in band_psum:
                        band_psum[pb] = psacc.tile(
                            [128, T], F32, tag="acc", name=f"acc{pb}")
                        band_done[pb] = 0
                    band_done[pb] += 1
                    nc.tensor.matmul(
                        band_psum[pb][:],
                        wt[p0:p1, t, :],
                        xt[p0:p1, t, :],
                        start=(band_done[pb] == 1),
                        stop=(band_done[pb] == band_nseg[pb]),
                    )
                    if band_done[pb] == band_nseg[pb]:
                        acc = band_psum.pop(pb)
                        # raw psum -> fp16 osb copy (scale/bias applied later)
                        if copy_alt[0] % 3 == 2:
                            nc.vector.tensor_copy(out=osb[:, pb, :], in_=acc[:])
                        else:
                            nc.scalar.activation(out=osb[:, pb, :], in_=acc[:],
                                                 func=AFT.Identity)
                        copy_alt[0] += 1

            def do_group(gi):
                g = GINFO[gi]
                b0, b1, t0, t1 = g["b0"], g["b1"], g["t0"], g["t1"]
                ng = b1 - b0
                ncol = t1 - t0
                # ---- per-(p,col) sum and sumsq proxies from bn_stats ----
                # s12m[...,0] = mean_e + mean_o          (=> col sum / 256)
                # s12m[...,1] = M2_e + M2_o + 256*(mean_e^2 + mean_o^2) (= col sumsq)
                me = s6[:, t0:t1, 1]
                mo = s6[:, t0:t1, 4]
                tmp = grp.tile([128, 32], F16, tag="tmp", name=f"tmp{gi}")
                tmp2 = grp.tile([128, 32], F16, tag="tmp2", name=f"tmp2{gi}")
                nc.vector.tensor_tensor(out=s12m[:, t0:t1, 0], in0=me, in1=mo,
                                        op=ALU.add)
                nc.vector.tensor_tensor(out=tmp[:, 0:ncol], in0=me, in1=me,
                                        op=ALU.mult)
                nc.vector.tensor_tensor(out=tmp2[:, 0:ncol], in0=mo, in1=mo,
                                        op=ALU.mult)
                nc.vector.tensor_tensor(out=tmp[:, 0:ncol], in0=tmp[:, 0:ncol],
                                        in1=tmp2[:, 0:ncol], op=ALU.add)
                nc.vector.tensor_scalar(out=tmp[:, 0:ncol], in0=tmp[:, 0:ncol],
                                        scalar1=256.0, scalar2=None,
                                        op0=ALU.mult)
                nc.vector.tensor_tensor(out=tmp2[:, 0:ncol],
                                        in0=s6[:, t0:t1, 2],
                                        in1=s6[:, t0:t1, 5], op=ALU.add)
                nc.vector.tensor_tensor(out=s12m[:, t0:t1, 1],
                                        in0=tmp[:, 0:ncol],
                                        in1=tmp2[:, 0:ncol], op=ALU.add)

                # ---- band aggregation: selector matmuls on PE ----
                # selector as stationary -> out [32, 2]: group-relative band
                # rows on partitions (base 0; all downstream slices base-0)
                sg = pssel.tile([32, 2], F32, tag="sel", name=f"sg{gi}")
                for k in range(g["k0"], g["k1"]):
                    _, t = SELCOLS[k]
                    nc.tensor.matmul(
                        sg[:],
                        selp[:, k, 0:32],
                        s12m[:, t, 0:2],
                        start=(k == g["k0"]),
                        stop=(k == g["k1"] - 1),
                    )

                # ---- tiny stats chain in band-partition layout ----
                mu = grp.tile([32, 1], F32, tag="mu", name=f"mu{gi}")
                var = grp.tile([32, 1], F32, tag="var", name=f"var{gi}")
                std = grp.tile([32, 1], F32, tag="std", name=f"std{gi}")
                # r in col 0 and r*mu in col 32, so a 32x32 block transpose
                # puts both on partition 0 (free 0:ng and 32:32+ng)
                rpk = grp.tile([32, 64], F32, tag="rpk", name=f"rpk{gi}")
                rT = grp.tile([32, 64], F32, tag="rT", name=f"rT{gi}")
                nc.vector.memset(rpk, 0.0)
                nc.vector.tensor_tensor(out=mu[0:ng, :],
                                        in0=sg[0:ng, 0:1],
                                        in1=cc[0:ng, 2 * gi:2 * gi + 1],
                                        op=ALU.mult)
                nc.vector.tensor_tensor(out=var[0:ng, :], in0=sg[0:ng, 1:2],
                                        in1=cc[0:ng, 2 * gi + 1:2 * gi + 2],
                                        op=ALU.mult)
                nc.vector.tensor_tensor(out=std[0:ng, :], in0=mu[0:ng, :],
                                        in1=mu[0:ng, :], op=ALU.mult)
                nc.vector.tensor_tensor(out=var[0:ng, :], in0=var[0:ng, :],
                                        in1=std[0:ng, :], op=ALU.subtract)
                nc.scalar.activation(out=std[0:ng, :], in_=var[0:ng, :],
                                     func=AFT.Sqrt, bias=epst[0:ng, 0:1])
                nc.vector.reciprocal(out=rpk[0:ng, 0:1], in_=std[0:ng, :])
                nc.vector.tensor_tensor(out=rpk[0:ng, 32:33], in0=rpk[0:ng, 0:1],
                                        in1=mu[0:ng, :], op=ALU.mult)
                nc.vector.transpose(out=rT, in_=rpk)
                rbbg = grp.tile([128, 64], F32, tag="rbb", name=f"rbb{gi}")
                bbvg = grp.tile([128, 32], F32, tag="bbv", name=f"bbv{gi}")
                nc.gpsimd.partition_broadcast(rbbg, rT[0:1, :])
                # bias vector: bbv = v - r*mu*u
                nc.vector.tensor_tensor(out=bbvg[:, 0:ng],
                                        in0=rbbg[:, 32:32 + ng],
                                        in1=uvb[:, 0, b0:b1], op=ALU.mult)
                nc.vector.tensor_tensor(out=bbvg[:, 0:ng],
                                        in0=uvb[:, 1, b0:b1],
                                        in1=bbvg[:, 0:ng], op=ALU.subtract)

                # ---- finalize bands in place, then ship the group ----
                for pb in range(b0, b1):
                    j = pb - b0
                    if j % 2 == 0:
                        nc.vector.tensor_scalar(
                            out=osb[:, pb, :], in0=osb[:, pb, :],
                            scalar1=rbbg[:, j:j + 1],
                            scalar2=bbvg[:, j:j + 1],
                            op0=ALU.mult, op1=ALU.add,
                        )
                    else:
                        nc.scalar.activation(
                            out=osb[:, pb, :], in_=osb[:, pb, :],
                            func=AFT.Identity,
                            scale=rbbg[:, j:j + 1],
                            bias=bbvg[:, j:j + 1],
                        )
                nc.gpsimd.dma_start(out=y_d[:, b0:b1, :], in_=osb[:, b0:b1, :])

            # ---------------- main emission loop over columns ----------------
            for t in range(N_COLS):
                if t in chunk_start:
                    c0, c1 = chunk_start[t]
                    eng = nc.sync if (c0 // 8) % 2 == 0 else nc.gpsimd
                    eng.dma_start(out=xt[:, c0:c1, :], in_=x_d[:, c0:c1, :])
                do_col(t)
                if t in gend:
                    do_group(gend[t])

    nc.finalize()
    return nc


_NC_CACHE = None


def _get_nc():
    global _NC_CACHE
    if _NC_CACHE is None:
        _NC_CACHE = _build_nc()
    return _NC_CACHE


def kernel(x, gamma, beta, W, b):
    from concourse.bass_utils import run_bass_kernel_spmd

    x = np.asarray(x, dtype=np.float32)
    gamma = np.asarray(gamma, dtype=np.float32)
    beta = np.asarray(beta, dtype=np.float32)
    W = np.asarray(W, dtype=np.float32)
    b = np.asarray(b, dtype=np.float32)

    wt, uvb, cc, sel = _pack_params(W, gamma, beta, b)
    xp = _pack_x(x)
    nc = _get_nc()
    in_maps = [
        {"xp": np.ascontiguousarray(xp[i]), "wt": wt, "sel": sel,
         "uvb": uvb, "cc": cc}
        for i in range(N_CORES)
    ]
    res = run_bass_kernel_spmd(nc, in_maps, list(range(N_CORES)))
    out = np.empty((N_CORES, OUT_CH, NBANDS, T), np.float32)
    for i in range(N_CORES):
        yp = res.results[i]["y"].astype(np.float32)   # packed band order
        for pb, bnat in enumerate(PACKED_BANDS):
            out[i, :, bnat, :] = yp[:, pb, :]
    return out
